# revision 1
# baseline (speedup 1.0000x reference)
"""MoE top-2 routing kernel for Trainium2 (8 NeuronCores, expert-parallel).

Strategy (per the expert-parallel sharding hint):
  1. Host computes the router exactly as the reference does (same jax ops)
     and derives each token's top-2 expert ids.
  2. The 2*N (token, expert) dispatch rows are sorted by expert and packed
     into a static per-core structure: `s` weight slots per core, slot j
     holding F_j 128-row tiles of a single expert. The structure (s, F_j)
     is chosen at runtime by an exact packer that minimizes tiles-per-core
     (the PE-time driver) and is baked into the compiled Bass program —
     all 8 cores run the identical program on different data (SPMD).
  3. On device, each core streams its slot weights into SBUF (bf16) and
     runs bf16 matmuls (full PE rate, fp32 PSUM accumulate, ~2e-3 rel err)
     over its tiles: out_tile[128, 1024] = x_tile[128, 1024] @ W_e[1024, 1024].
  4. Host gathers the per-core partials and combines: for each token,
     out = 0.5 * (f_e1(x) + f_e2(x)) with the expert biases added on host.

bf16 halves HBM traffic vs fp32 (the DMA floor drops well below the PE
floor), so the kernel is tensor-bound: ~17 tiles/core x 3.5us/tile.
"""

import os
from contextlib import ExitStack
from functools import lru_cache

import ml_dtypes
import numpy as np

import concourse.bass as bass
import concourse.mybir as mybir
import concourse.tile as tile
from concourse import bacc
from concourse.bass_utils import run_bass_kernel_spmd

N = 8192
D = 1024
E = 10
TOP_K = 2
P = 128
KC = D // P  # 8 contraction chunks of 128
NCORES = 8
BF16 = ml_dtypes.bfloat16

_last_results = None  # stash for test harness (exec_time_ns etc.)


def _route(x, Wr, br):
    """Top-2 expert ids per token, replicating reference ops exactly."""
    import jax
    import jax.numpy as jnp

    logits = jnp.asarray(x) @ jnp.asarray(Wr).T + jnp.asarray(br)
    probs = jax.nn.softmax(logits, axis=-1)
    _, idx = jax.lax.top_k(probs, TOP_K)
    return np.asarray(idx)


def _pack(tiles_per_expert):
    """Choose slot sizes (shared by all cores) and assign expert tile-pieces
    to (core, slot) cells. Minimizes tiles-per-core (PE time), then slot
    count (weight DMA). Exact search: a cell (core, slot j) holds up to
    sizes[j] tiles of ONE expert; total overcapacity across all 8*s cells
    is bounded by 8*tpc - total_tiles, so nearly every cell must be filled
    exactly — a small memoized DFS settles feasibility fast.
    Returns (sizes, assign) with assign a list of (expert, core, slot, n_tiles).
    """
    import time

    active = [(t, e) for e, t in enumerate(tiles_per_expert) if t > 0]
    total = sum(t for t, _ in active)
    tmax = max(t for t, _ in active)
    base = -(-total // NCORES)  # ceil
    deadline = time.time() + 15.0

    def parts(tot, k, maxv):
        """Descending partitions of tot into exactly k parts, each <= maxv."""
        if k == 1:
            if 1 <= tot <= maxv:
                yield (tot,)
            return
        for a in range(min(tot - k + 1, maxv), (tot - 1) // k, -1):
            for rest in parts(tot - a, k - 1, a):
                yield (a,) + rest

    def solve(shape):
        """Find an exact piece assignment for cell shape `shape` (x8 cores),
        or None. Pieces: (expert, cell_size_class, n_tiles)."""
        sizes_desc = sorted(set(shape), reverse=True)
        counts = tuple(shape.count(sz) * NCORES for sz in sizes_desc)
        budget = NCORES * sum(shape) - total
        nodes = [0]

        @lru_cache(maxsize=None)
        def dfs(needs, avail, slack):
            if not needs:
                return ()
            if nodes[0] > 200_000:
                return None
            nodes[0] += 1
            n, e = needs[0]
            rest = needs[1:]
            for i, sz in enumerate(sizes_desc):
                if avail[i] == 0:
                    continue
                waste = sz - n if n <= sz else 0
                if waste > slack:
                    continue
                new_avail = avail[:i] + (avail[i] - 1,) + avail[i + 1 :]
                if n > sz:
                    new_needs = tuple(
                        sorted(rest + ((n - sz, e),), reverse=True)
                    )
                else:
                    new_needs = rest
                sub = dfs(new_needs, new_avail, slack - waste)
                if sub is not None:
                    return ((e, sz, min(n, sz)),) + sub
            return None

        needs0 = tuple(sorted(((t, e) for t, e in active), reverse=True))
        return dfs(needs0, counts, budget)

    def realize(shape, pieces):
        sizes = list(shape)
        cells_by_size = {}
        for j, sz in enumerate(sizes):
            for c in range(NCORES):
                cells_by_size.setdefault(sz, []).append((c, j))
        assign = []
        for e, sz, take in pieces:
            c, j = cells_by_size[sz].pop(0)
            assign.append((e, c, j, take))
        return sizes, assign

    expired = False
    for tpc in range(base, base + 8):
        for s in (1, 2, 3, 4):
            for shape in parts(tpc, s, tpc):
                if time.time() > deadline:
                    expired = True
                    break
                pieces = solve(shape)
                if pieces is not None:
                    return realize(shape, pieces)
            if expired:
                break
        if expired:
            break

    # backstop: always feasible (each expert whole in one cell)
    shape = (tmax, tmax)
    cells = [(c, j) for j in range(2) for c in range(NCORES)]
    assign = [
        (e, *cells[k], t) for k, (t, e) in enumerate(sorted(active, reverse=True))
    ]
    return list(shape), assign


def _build_program_raw(sizes):
    """Hand-scheduled raw-bass SPMD program (no TileContext): engine
    streams with explicit semaphores. Avoids Tile's ~10us exit chain of
    per-semaphore resets and gives exact DMA ordering control.

    Layout per core: `s` weight slots (bf16, KC chunk tiles each), tpc
    x-tiles streamed through XB buffers, fp32 PSUM accumulate, bf16 out.
    Matmuls run kk-major (each 128-wide x chunk loaded once as the
    stationary operand feeds two N=512 matmuls — LDWEIGHTS fully hides,
    216ns/MM steady state). All load/store DMA runs on HWDGE queues
    (sync: x + out, scalar: w); gpsimd only clears semaphores up front,
    so its slow SWDGE drain happens during the stream, not at the tail."""
    s = len(sizes)
    tpc = sum(sizes)
    bf16 = mybir.dt.bfloat16
    f32 = mybir.dt.float32
    XB, OB, PB = 6, 6, 3  # x tiles, out chunk bufs, psum tiles in flight
    WBUF = min(3, len(sizes))  # weight slot buffers in flight
    H = 2  # psum halves per tile (512 cols each)
    WARM_MM = int(os.environ.get("KERNEL_WARM", "58"))  # sized to
    # bridge the PE from block start (~7.5us) to first-supply (~14us) so the
    # HAM clock gate never re-throttles before the real stream begins

    # slot id / first-tile flag for each global tile index
    slot_of = []
    first_of_slot = []
    for j, F in enumerate(sizes):
        for i in range(F):
            slot_of.append(j)
            first_of_slot.append(i == 0)
    slot_end = np.cumsum(sizes).tolist()  # tiles completed at end of slot j

    # Copy/store chunks per tile: halves on the vector engine normally;
    # the last tile is cast as four quarters so the final
    # copy->store->receipt chain is shorter. Entry:
    # (lane_idx, col_off, width, engine, per_engine_seq)
    chunks = []
    gi = 0
    nv = ns = 0
    for t in range(tpc):
        cl = []
        # Halves everywhere, all on vector: the DVE cast path has no
        # one-time table load (scalar ACTIVATE pays ~1.3us ACT_TABLE_LOAD
        # at first use), and HWDGE per-op issue (~0.6us) makes finer
        # store granularity a net loss.
        plan = [(0, 512, 'v'), (512, 512, 'v')]
        for off, wid, eng in plan:
            if eng == 'v':
                nv += 1
                seq = nv
            else:
                ns += 1
                seq = ns
            cl.append((gi, off, wid, eng, seq))
            gi += 1
        chunks.append(cl)
    n_chunks = gi
    # vector-chunks completed through tile t (for PSUM-reuse gating; the
    # last tile's PSUM is never reused so its split doesn't matter here)
    c_end = np.cumsum(
        [sum(1 for c in cl if c[3] == 'v') for cl in chunks]
    ).tolist()

    pair = (
        sizes[0] >= 2 and tpc > 6
        and os.environ.get("KERNEL_PAIR", "1") == "1"
    )
    nc = bass.Bass("TRN2", target_bir_lowering=False, debug=False)
    xT = nc.dram_tensor("xT", [tpc, P, D], bf16, kind="ExternalInput")
    if pair:
        # tiles 0+1 side by side: 4KB contiguous per partition row, so the
        # one 512KB head transfer uses 4KB descriptors (~2x the rate of
        # the 2KB-descriptor [P, D] tile loads)
        xT01 = nc.dram_tensor("xT01", [P, 2 * D], bf16, kind="ExternalInput")
    w = nc.dram_tensor("w", [s, P, KC * D], bf16, kind="ExternalInput")
    out = nc.dram_tensor("out", [tpc, P, D], bf16, kind="ExternalOutput")

    with ExitStack() as ctx:
        xb = [
            ctx.enter_context(nc.sbuf_tensor(f"xb{i}", [P, D], bf16))
            for i in range(XB)
        ]
        wb = [
            [
                ctx.enter_context(nc.sbuf_tensor(f"wb{b}_{c}", [P, D], bf16))
                for c in range(KC)
            ]
            for b in range(WBUF)
        ]
        ob = [
            ctx.enter_context(nc.sbuf_tensor(f"ob{i}", [P, 512], bf16))
            for i in range(OB)
        ]
        if pair:
            xb01 = ctx.enter_context(nc.sbuf_tensor("xb01", [P, 2 * D], bf16))
        # Warm-up operands are never initialized: the PE computes on
        # whatever SBUF holds (results land in pwarm, never read), so the
        # warm matmuls can start immediately with no memset dependency.
        # bf16 so the warm matmuls use the fast single-pass PE mode.
        warm = ctx.enter_context(nc.sbuf_tensor("warmt", [P, P + P], bf16))
        actwarm = ctx.enter_context(nc.sbuf_tensor("actwarm", [P, 512], bf16))
        pb = [
            ctx.enter_context(nc.psum_tensor(f"pb{i}", [P, D], f32))
            for i in range(PB)
        ]
        pwarm = ctx.enter_context(nc.psum_tensor("pwarm", [P, P], f32))
        # Completion semaphores rotate per stream (lane = index mod depth):
        # increments of consecutive DMAs on one queue can interleave, so a
        # cumulative count on a single sem may pass a wait while the newest
        # transfer is still partially in flight. With one DMA in flight per
        # lane the per-lane cumulative count is sound.
        sem_x = [
            ctx.enter_context(nc.semaphore(f"sem_x{i}")) for i in range(XB)
        ]
        sem_w = [
            ctx.enter_context(nc.semaphore(f"sem_w{i}")) for i in range(KC)
        ]
        sem_out = [
            ctx.enter_context(nc.semaphore(f"sem_o{i}")) for i in range(OB)
        ]
        sem_mm = ctx.enter_context(nc.semaphore("sem_mm"))  # counts tiles
        sem_cp = ctx.enter_context(nc.semaphore("sem_cp"))  # vector casts
        sem_cs = ctx.enter_context(nc.semaphore("sem_cs"))  # scalar casts

        # Prior programs (e.g. XLA executables) leave semaphores dirty.
        # Clear ours on gpsimd while every engine holds at an NRT-level
        # pseudo-barrier (safe before bass sems are valid), then start.
        sems = sem_x + sem_w + sem_out + [sem_mm, sem_cp, sem_cs]
        nums = sorted(sm.num for sm in sems)
        nc.gpsimd.dma_reset(range(nums[0], nums[-1] + 1))
        # First two transfers ride SWDGE right behind the reset, before the
        # other engines clear the barrier — the data is in flight ~2us
        # earlier than a post-barrier HWDGE issue could manage. (Only two:
        # Q7 descriptor emission is ~1us per op, and the gpsimd dge-drain
        # this incurs runs mid-stream, off the critical path.)
        preb = (
            os.environ.get("KERNEL_PREB", "0") == "1" and not pair
        )
        if preb:
            nc.gpsimd.dma_start(out=xb[0][:], in_=xT[0]).then_inc(
                sem_x[0], 16
            )
            nc.gpsimd.dma_start(
                out=wb[0][0][:], in_=w[0, :, 0:D]
            ).then_inc(sem_w[0], 16)
        nc._nrt_pseudo_barrier()

        block = ctx.enter_context(nc.Block())

        @block.sync
        def _(sync):
            # Head, in first-use order on ONE ring so the first tiles are
            # fed as fast as possible: x tile 1, then slot-0 weight chunks
            # 1..7 (x0/w0 already in flight via gpsimd), then the deeper
            # x prefetches.
            if pair:
                sync.dma_start(out=xb01[:], in_=xT01[:]).then_inc(
                    sem_x[0], 16
                )
                sync.dma_start(
                    out=wb[0][0][:], in_=w[0, :, 0:D]
                ).then_inc(sem_w[0], 16)
            else:
                if not preb:
                    sync.dma_start(out=xb[0][:], in_=xT[0]).then_inc(
                        sem_x[0], 16
                    )
                    sync.dma_start(
                        out=wb[0][0][:], in_=w[0, :, 0:D]
                    ).then_inc(sem_w[0], 16)
                if tpc > 1:
                    sync.dma_start(out=xb[1][:], in_=xT[1]).then_inc(
                        sem_x[1], 16
                    )
            for c in range(1, KC):
                sync.dma_start(
                    out=wb[0][c][:], in_=w[0, :, c * D : (c + 1) * D]
                ).then_inc(sem_w[c], 16)
            for t in range(2, min(XB, tpc)):
                sync.dma_start(out=xb[t % XB][:], in_=xT[t]).then_inc(
                    sem_x[t % XB], 16
                )
            for t in range(tpc):
                # prefetch x tile t+XB, then store tile t's chunks —
                # matches readiness order (tile t's casts trail its
                # matmuls, which gate the prefetch).
                if t + XB < tpc:
                    sync.wait_ge(sem_mm, t + 1)
                    sync.dma_start(
                        out=xb[(t + XB) % XB][:], in_=xT[t + XB]
                    ).then_inc(sem_x[(t + XB) % XB], 16)
                for g, off, wid, eng, seq in chunks[t]:
                    sync.wait_ge(sem_cp if eng == 'v' else sem_cs, seq)
                    sync.dma_start(
                        out=out[t, :, off : off + wid], in_=ob[g % OB][:, :wid]
                    ).then_inc(sem_out[g % OB], 16)
            for l in range(OB):
                uses = (n_chunks - l + OB - 1) // OB
                sync.wait_ge(sem_out[l], 16 * uses)

        @block.scalar
        def _(scalar):
            for j in range(1, s):
                # Touch the activation path once mid-stream (same
                # PSUM-f32 -> SBUF-bf16 form as the tail casts) so the
                # one-time ACT_TABLE_LOAD (~1.3us) is off the tail's
                # critical path. pwarm is sem-ordered retired: the j>=2
                # gate implies many tiles done, so the warm matmuls that
                # wrote it are long finished.
                if j == 2 and os.environ.get("KERNEL_ACTWARM", "0") == "1":
                    nc.scalar.copy(out=actwarm[:, 0:P], in_=pwarm[:, 0:P])
                # gate >= 1 also orders this slot's sem_w[0] increment
                # after slot 0 chunk 0's (tile 0 done implies it landed)
                gate = 1
                if j >= 1:
                    # don't steal head HBM bandwidth from slot 0 / x tiles;
                    # a 4-tile window (~14us) comfortably covers the 2MB load
                    gate = max(gate, max(0, slot_end[j - 1] - 4))
                if j >= WBUF:
                    # slot j reuses slot j-WBUF's buffer
                    gate = max(gate, slot_end[j - WBUF])
                if gate:
                    scalar.wait_ge(sem_mm, gate)
                for c in range(KC):
                    scalar.dma_start(
                        out=wb[j % WBUF][c][:], in_=w[j, :, c * D : (c + 1) * D]
                    ).then_inc(sem_w[c], 16)
            # last tile: cast any scalar-assigned quarters in parallel
            # with the vector engine's to shorten the tail
            if any(c[3] == 's' for c in chunks[tpc - 1]):
                scalar.wait_ge(sem_mm, tpc)
            for g, off, wid, eng, seq in chunks[tpc - 1]:
                if eng != 's':
                    continue
                if g >= OB:
                    scalar.wait_ge(sem_out[g % OB], 16 * (g // OB))
                nc.scalar.copy(
                    out=ob[g % OB][:, :wid], in_=pb[(tpc - 1) % PB][:, off : off + wid]
                ).then_inc(sem_cs, 1)

        @block.tensor
        def _(tensor):
            # Warm-up: garbage matmuls keep the PE busy through the DMA
            # head so the HAM clock gate (1.2->2.4GHz) releases by the
            # time real matmuls start.
            for _ in range(WARM_MM):
                nc.tensor.matmul(
                    pwarm[:], warm[:, :P], warm[:, P : P + P],
                    start=True, stop=True,
                )
            # Tiles 0 and 1 interleaved per weight chunk: the head is
            # weight-DMA-paced (~0.73us/chunk), so giving each arriving
            # chunk two tiles' worth of matmuls (~0.86us) keeps the PE
            # busy instead of idling between chunks.
            t_start = 0
            if pair:
                t_start = 2
                tensor.wait_ge(sem_x[0], 16)
                for kk in range(KC):
                    tensor.wait_ge(sem_w[kk], 16)
                    for tt in (0, 1):
                        for nh in range(H):
                            mm = nc.tensor.matmul(
                                pb[tt][:, nh * 512 : (nh + 1) * 512],
                                xb01[:, tt * D + kk * P : tt * D + (kk + 1) * P],
                                wb[0][kk][:, nh * 512 : (nh + 1) * 512],
                                start=(kk == 0),
                                stop=(kk == KC - 1),
                            )
                        if kk == KC - 1:
                            mm.then_inc(sem_mm, 1)
            for t in range(t_start, tpc):
                j = slot_of[t]
                ps = pb[t % PB]
                if pair:
                    # sem'd x loads start at x2 (x0/x1 rode xb01)
                    tensor.wait_ge(sem_x[t % XB], 16 * ((t - 2) // XB + 1))
                else:
                    tensor.wait_ge(sem_x[t % XB], 16 * (t // XB + 1))
                if t >= PB:
                    tensor.wait_ge(sem_cp, c_end[t - PB])
                for kk in range(KC):
                    if first_of_slot[t]:
                        # later tiles of the slot are covered by program
                        # order on this queue
                        tensor.wait_ge(sem_w[kk], 16 * (j + 1))
                    for nh in range(H):
                        mm = nc.tensor.matmul(
                            ps[:, nh * 512 : (nh + 1) * 512],
                            xb[t % XB][:, kk * P : (kk + 1) * P],
                            wb[j % WBUF][kk][:, nh * 512 : (nh + 1) * 512],
                            start=(kk == 0),
                            stop=(kk == KC - 1),
                        )
                mm.then_inc(sem_mm, 1)

        @block.vector
        def _(vector):
            for t in range(tpc):
                for g, off, wid, eng, seq in chunks[t]:
                    if eng != 'v':
                        continue
                    vector.wait_ge(sem_mm, t + 1)
                    if g >= OB:
                        vector.wait_ge(sem_out[g % OB], 16 * (g // OB))
                    nc.vector.tensor_copy(
                        ob[g % OB][:, :wid], pb[t % PB][:, off : off + wid]
                    ).then_inc(sem_cp, 1)

    return nc


def _build_program(sizes):
    """Tile-scheduled fallback (KERNEL_IMPL=tile): same structure, Tile
    framework handles semaphores. Carries ~10us of exit-chain overhead."""
    s = len(sizes)
    nc = bacc.Bacc("TRN2", target_bir_lowering=False, debug=False)
    bf16 = mybir.dt.bfloat16
    f32 = mybir.dt.float32
    tpc = sum(sizes)

    xT = nc.dram_tensor("xT", [tpc, P, D], bf16, kind="ExternalInput")
    w = nc.dram_tensor("w", [s, P, KC * D], bf16, kind="ExternalInput")
    out = nc.dram_tensor("out", [tpc, P, D], bf16, kind="ExternalOutput")

    with tile.TileContext(nc) as tc:
        with (
            tc.tile_pool(name="cp", bufs=1) as cp,
            tc.tile_pool(name="wp", bufs=2) as wp,
            tc.tile_pool(name="xp", bufs=6) as xp,
            tc.tile_pool(name="op", bufs=6) as op,
            tc.tile_pool(name="pp", bufs=3, space="PSUM") as pp,
            tc.tile_pool(name="wmp", bufs=1, space="PSUM") as wmp,
        ):
            # PE warm-up: small matmuls with no data deps run during the
            # initial DMA head, releasing the HAM clock gate (1.2->2.4GHz)
            # before the real matmuls start.
            wc = cp.tile([P, P + P], f32, tag="warm")
            nc.gpsimd.memset(wc[:], 0.0)
            wps = wmp.tile([P, P], f32, tag="warmps")
            for _ in range(18):
                nc.tensor.matmul(
                    wps[:], wc[:, :P], wc[:, P : P + P], start=True, stop=True
                )
            t_global = 0
            for j in range(s):
                # weight slot as KC separate chunk tiles so matmul kk
                # only waits for its own chunk's DMA (issued on scalar queue)
                wts = []
                for kk in range(KC):
                    wt = wp.tile([P, D], bf16, tag=f"w{kk}")
                    nc.scalar.dma_start(
                        out=wt[:], in_=w[j, :, kk * D : (kk + 1) * D]
                    )
                    wts.append(wt)
                for _ in range(sizes[j]):
                    xt = xp.tile([P, D], bf16, tag="x")
                    nc.sync.dma_start(out=xt[:], in_=xT[t_global])
                    ps = pp.tile([P, D], f32, tag="ps")
                    for kk in range(KC):
                        lhsT = xt[:, kk * P : (kk + 1) * P]
                        for nh in range(2):
                            nc.tensor.matmul(
                                ps[:, nh * 512 : (nh + 1) * 512],
                                lhsT,
                                wts[kk][:, nh * 512 : (nh + 1) * 512],
                                start=(kk == 0),
                                stop=(kk == KC - 1),
                            )
                    # half-tile copyback + store overlaps copy of one half
                    # with the DMA of the other; shortens the kernel tail
                    for nh in range(2):
                        ot = op.tile([P, 512], bf16, tag="o")
                        nc.vector.tensor_copy(ot[:], ps[:, nh * 512 : (nh + 1) * 512])
                        nc.gpsimd.dma_start(
                            out=out[t_global, :, nh * 512 : (nh + 1) * 512],
                            in_=ot[:],
                        )
                    t_global += 1
    nc.compile()
    return nc


def kernel(x, Wr, br, We, be):
    global _last_results
    x = np.ascontiguousarray(np.asarray(x, dtype=np.float32))
    Wr = np.asarray(Wr, dtype=np.float32)
    br = np.asarray(br, dtype=np.float32)
    We = np.asarray(We, dtype=np.float32)
    be = np.asarray(be, dtype=np.float32)

    idx = _route(x, Wr, br)  # [N, 2] int32

    # token lists per expert (sorted by token id)
    token_lists = [np.nonzero((idx == e).any(axis=1))[0] for e in range(E)]
    tiles_per_expert = [-(-len(t) // P) for t in token_lists]
    sizes, assign = _pack(tiles_per_expert)
    s, tpc = len(sizes), sum(sizes)
    slot_off = np.concatenate([[0], np.cumsum(sizes)])  # tile offset of slot j

    x_bf = x.astype(BF16)
    # weights for a slot: [ki, kk*D + n] = We[e, kk*P + ki, n]
    wT_cache = {}

    def wT(e):
        if e not in wT_cache:
            wT_cache[e] = (
                We[e].astype(BF16).reshape(KC, P, D).transpose(1, 0, 2)
                .reshape(P, KC * D)
            )
        return wT_cache[e]

    # Build per-core inputs + bookkeeping
    xT_cores = np.zeros((NCORES, tpc, P, D), dtype=BF16)
    w_cores = np.zeros((NCORES, s, P, KC * D), dtype=BF16)
    pos = np.full((N, TOP_K), -1, dtype=np.int64)
    exp = np.full((N, TOP_K), -1, dtype=np.int64)
    cnt = np.zeros(N, dtype=np.int64)

    taken = [0] * E  # tiles of expert e already dispatched
    for e, c, j, ntiles in assign:
        toks_all = token_lists[e]
        start = taken[e] * P
        stop = min(start + ntiles * P, len(toks_all))
        taken[e] += ntiles
        toks = toks_all[start:stop]
        nrow = len(toks)
        w_cores[c, j] = wT(e)
        if nrow == 0:
            continue
        nt_used = -(-nrow // P)
        xs = np.zeros((nt_used * P, D), dtype=BF16)
        xs[:nrow] = x_bf[toks]
        # per tile: [ki, kk*P + m] = xs[tile*P + m, kk*P + ki]
        blk = xs.reshape(nt_used, P, KC, P).transpose(0, 3, 2, 1).reshape(
            nt_used, P, D
        )
        t0 = slot_off[j]
        xT_cores[c, t0 : t0 + nt_used] = blk
        # flat row positions in the concatenated [NCORES * tpc * P] output
        flat = c * (tpc * P) + t0 * P + np.arange(nrow)
        pos[toks, cnt[toks]] = flat
        exp[toks, cnt[toks]] = e
        cnt[toks] += 1

    assert (cnt == TOP_K).all(), "dispatch did not cover every token twice"

    if os.environ.get("KERNEL_IMPL", "raw") == "raw":
        nc = _build_program_raw(sizes)
    else:
        nc = _build_program(sizes)
    pair = (
        sizes[0] >= 2 and tpc > 6
        and os.environ.get("KERNEL_PAIR", "1") == "1"
    )
    in_maps = [{"xT": xT_cores[c], "w": w_cores[c]} for c in range(NCORES)]
    if pair and os.environ.get("KERNEL_IMPL", "raw") == "raw":
        for c in range(NCORES):
            in_maps[c]["xT01"] = np.ascontiguousarray(
                np.concatenate([xT_cores[c, 0], xT_cores[c, 1]], axis=1)
            )
    res = run_bass_kernel_spmd(nc, in_maps, core_ids=list(range(NCORES)))
    _last_results = res

    partial = np.concatenate(
        [
            res.results[c]["out"].reshape(tpc * P, D).astype(np.float32)
            for c in range(NCORES)
        ]
    )
    out = 0.5 * (partial[pos[:, 0]] + partial[pos[:, 1]]) + 0.5 * (
        be[exp[:, 0]] + be[exp[:, 1]]
    )
    return out.astype(np.float32)



# revision 2
# speedup vs baseline: 1.1514x; 1.1514x over previous
"""MoE top-2 routing kernel for Trainium2 (8 NeuronCores, merged-pair).

Key algebraic trick: the reference combine is an UNWEIGHTED mean of the
two selected experts, so for every token
    out = 0.5*(x @ We1 + x @ We2) + 0.5*(be1 + be2)
        = x @ (0.5*(We1 + We2)) + 0.5*(be1 + be2).
Tokens sharing the same top-2 pair (45 distinct pairs for E=10) need only
ONE matmul against the host-pre-merged pair weight — half the PE work of
per-expert dispatch. The kernel is then DMA-bound on streaming the ~2MB
bf16 merged weight per pair (45 x 2MB over 8 cores ~ 12MB/core).

Orientation: "transposed" streaming. Stationary operand = 128x128 W
blocks in natural [K, F] layout; moving operand = x^T columns (tokens).
PE cost is 64*T cycles per pair (T = token count) with NO padding to
128-token tiles, and x/out DMA carry no padding either.

SPMD: one program for all 8 cores, so the slot structure is rank-uniform:
S=6 slots per core (45 pairs + 3 split halves = 48 pieces, snake-dealt
by size), rank r padded to a common width prof[r] across cores.

Per core: 6 weight slots (2MB each, loaded as 2 half-DMAs on the sync
HWDGE queue), one x^T tensor (~2.2MB, 2 DMAs on scalar queue), psum
bank f per f-chunk, DVE casts psum->bf16, per-slot out stores (scalar).
Host does routing, merging, packing, gather/scatter, bias add.
"""

import os
from contextlib import ExitStack

import ml_dtypes
import numpy as np

import concourse.bass as bass
import concourse.mybir as mybir
from concourse.bass_utils import run_bass_kernel_spmd

N = 8192
D = 1024
E = 10
TOP_K = 2
P = 128
KC = 8   # contraction chunks of 128
FC = 8   # output-feature chunks of 128
NCORES = 8
BF16 = ml_dtypes.bfloat16

_last_results = None  # stash for test harness (exec_time_ns etc.)
_prog_cache = {}


def _route(x, Wr, br):
    """Top-2 expert ids per token, replicating reference ops exactly."""
    import jax
    import jax.numpy as jnp

    logits = jnp.asarray(x) @ jnp.asarray(Wr).T + jnp.asarray(br)
    probs = jax.nn.softmax(logits, axis=-1)
    _, idx = jax.lax.top_k(probs, TOP_K)
    return np.asarray(idx)


def _pack(pieces):
    """pieces: list of (pid, tok_array). Split/pad to exactly 8*S pieces
    (S >= ceil/8), snake-deal sorted-desc into an 8 x S grid, and return
    (grid, prof): grid[c][r] = (pid, toks), prof[r] = common padded width
    of rank r (multiple of 4, >= 4, <= 512)."""
    pieces = [(pid, t) for pid, t in pieces if len(t) > 0]
    # psum bank limit: T <= 512
    changed = True
    while changed:
        changed = False
        for i, (pid, t) in enumerate(pieces):
            if len(t) > 512:
                h = len(t) // 2
                pieces[i] = (pid, t[:h])
                pieces.append((pid, t[h:]))
                changed = True
    S = max(1, -(-len(pieces) // 8))
    while len(pieces) < 8 * S:
        pieces.sort(key=lambda p: -len(p[1]))
        pid, t = pieces[0]
        if len(t) >= 2:
            h = len(t) // 2
            pieces[0] = (pid, t[:h])
            pieces.append((pid, t[h:]))
        else:
            pieces.append((-1, np.zeros(0, dtype=np.int64)))
    pieces.sort(key=lambda p: -len(p[1]))
    grid = [[None] * S for _ in range(NCORES)]
    for r in range(S):
        row = pieces[8 * r : 8 * r + 8]
        order = range(NCORES) if r % 2 == 0 else range(NCORES - 1, -1, -1)
        for k, c in enumerate(order):
            grid[c][r] = row[k]
    prof = []
    for r in range(S):
        mx = max(len(grid[c][r][1]) for c in range(NCORES))
        prof.append(max(4, -(-mx // 4) * 4))
    return grid, prof


def _build_program(prof):
    """Raw-bass SPMD program: S weight slots of common rank widths prof.

    Engines: sync = weight half-DMAs (HWDGE), scalar = x loads + out
    stores (HWDGE), tensor = warmup + 8 f-groups x 8 ki matmuls per slot,
    vector = psum->sbuf bf16 casts, gpsimd = semaphore reset up front.
    """
    S = len(prof)
    sumT = sum(prof)
    Xoff = [0]
    for t in prof:
        Xoff.append(Xoff[-1] + 8 * t)
    bf16 = mybir.dt.bfloat16
    f32 = mybir.dt.float32
    WBUF = min(3, S)
    OB = min(3, S)
    WARM = int(os.environ.get("KERNEL_WARM", "38"))

    nc = bass.Bass("TRN2", target_bir_lowering=False, debug=False)
    xT = nc.dram_tensor("xT", [P, 8 * sumT], bf16, kind="ExternalInput")
    w = nc.dram_tensor("w", [S, P, KC * D], bf16, kind="ExternalInput")
    out = nc.dram_tensor("out", [P, 8 * sumT], bf16, kind="ExternalOutput")

    with ExitStack() as ctx:
        xb = ctx.enter_context(nc.sbuf_tensor("xb", [P, 8 * sumT], bf16))
        wb = [
            ctx.enter_context(nc.sbuf_tensor(f"wb{b}", [P, KC * D], bf16))
            for b in range(WBUF)
        ]
        ob = [
            ctx.enter_context(nc.sbuf_tensor(f"ob{i}", [P, 8 * prof[0]], bf16))
            for i in range(OB)
        ]
        # Warm-up operands are never initialized: the PE computes on
        # whatever SBUF holds; results land in pb[7] and are reset by the
        # first real f=7 accumulation group (start=True).
        warm = ctx.enter_context(nc.sbuf_tensor("warmt", [P, 2 * P], bf16))
        pb = [
            ctx.enter_context(nc.psum_tensor(f"pb{i}", [P, 512], f32))
            for i in range(8)
        ]
        sem_x = [ctx.enter_context(nc.semaphore(f"sem_x{i}")) for i in range(2)]
        sem_w = [
            ctx.enter_context(nc.semaphore(f"sem_w{i}")) for i in range(WBUF)
        ]
        sem_o = [
            ctx.enter_context(nc.semaphore(f"sem_o{i}")) for i in range(OB)
        ]
        sem_mm = ctx.enter_context(nc.semaphore("sem_mm"))  # f-groups done
        sem_cp = ctx.enter_context(nc.semaphore("sem_cp"))  # vector casts

        sems = sem_x + sem_w + sem_o + [sem_mm, sem_cp]
        nums = sorted(sm.num for sm in sems)
        nc.gpsimd.dma_reset(range(nums[0], nums[-1] + 1))
        nc._nrt_pseudo_barrier()

        block = ctx.enter_context(nc.Block())

        @block.sync
        def _(sync):
            # Weight stream: slot j as two 1MB half-DMAs (f-chunks 0-3 /
            # 4-7) so the PE can start a slot's first f-groups one half
            # earlier. Buffer reuse gated on the consuming slot's last
            # f-group.
            for j in range(S):
                if j >= WBUF:
                    sync.wait_ge(sem_mm, 8 * (j - WBUF + 1))
                for h in range(2):
                    sync.dma_start(
                        out=wb[j % WBUF][:, h * 4096 : (h + 1) * 4096],
                        in_=w[j, :, h * 4096 : (h + 1) * 4096],
                    ).then_inc(sem_w[j % WBUF], 16)
            for l in range(OB):
                uses = (S - l + OB - 1) // OB
                if uses > 0:
                    sync.wait_ge(sem_o[l], 16 * uses)

        @block.scalar
        def _(scalar):
            # x^T: rank-0 slice first (unblocks slot 0), then the rest.
            scalar.dma_start(
                out=xb[:, 0 : 8 * prof[0]], in_=xT[:, 0 : 8 * prof[0]]
            ).then_inc(sem_x[0], 16)
            if S > 1:
                scalar.dma_start(
                    out=xb[:, 8 * prof[0] :], in_=xT[:, 8 * prof[0] :]
                ).then_inc(sem_x[1], 16)
            for j in range(S):
                scalar.wait_ge(sem_cp, 8 * (j + 1))
                wj = 8 * prof[j]
                scalar.dma_start(
                    out=out[:, Xoff[j] : Xoff[j] + wj], in_=ob[j % OB][:, :wj]
                ).then_inc(sem_o[j % OB], 16)

        @block.tensor
        def _(tensor):
            # Garbage warm-up matmuls bridge the DMA head so the HAM
            # clock gate (1.2->2.4GHz) is released when real work starts.
            for _ in range(WARM):
                nc.tensor.matmul(
                    pb[7][:, 0:P], warm[:, :P], warm[:, P : 2 * P],
                    start=True, stop=True,
                )
            for j in range(S):
                Tj = prof[j]
                for f in range(FC):
                    if f == 0:
                        if j <= 1:
                            tensor.wait_ge(sem_x[min(j, 1)], 16)
                        tensor.wait_ge(sem_w[j % WBUF], 32 * (j // WBUF) + 16)
                    if f == 4:
                        tensor.wait_ge(sem_w[j % WBUF], 32 * (j // WBUF) + 32)
                    if j >= 1:
                        # psum bank f reused from slot j-1: wait for its cast
                        tensor.wait_ge(sem_cp, 8 * (j - 1) + f + 1)
                    for kk in range(KC):
                        mm = nc.tensor.matmul(
                            pb[f][:, 0:Tj],
                            wb[j % WBUF][
                                :, f * 1024 + kk * 128 : f * 1024 + (kk + 1) * 128
                            ],
                            xb[:, Xoff[j] + kk * Tj : Xoff[j] + (kk + 1) * Tj],
                            start=(kk == 0),
                            stop=(kk == KC - 1),
                        )
                    mm.then_inc(sem_mm, 1)

        @block.vector
        def _(vector):
            for j in range(S):
                Tj = prof[j]
                for f in range(FC):
                    vector.wait_ge(sem_mm, 8 * j + f + 1)
                    if j >= OB and f == 0:
                        vector.wait_ge(sem_o[j % OB], 16 * (j // OB))
                    nc.vector.tensor_copy(
                        ob[j % OB][:, f * Tj : (f + 1) * Tj], pb[f][:, 0:Tj]
                    ).then_inc(sem_cp, 1)

    return nc


def kernel(x, Wr, br, We, be):
    global _last_results
    x = np.ascontiguousarray(np.asarray(x, dtype=np.float32))
    Wr = np.asarray(Wr, dtype=np.float32)
    br = np.asarray(br, dtype=np.float32)
    We = np.asarray(We, dtype=np.float32)
    be = np.asarray(be, dtype=np.float32)

    idx = _route(x, Wr, br)  # [N, 2] int32
    pr = np.sort(idx, axis=1)
    pid_tok = pr[:, 0] * E + pr[:, 1]  # pair id per token

    order = np.argsort(pid_tok, kind="stable")
    pids, starts = np.unique(pid_tok[order], return_index=True)
    tok_lists = np.split(order, starts[1:])
    pieces = list(zip(pids.tolist(), tok_lists))

    grid, prof = _pack(pieces)
    S = len(prof)
    sumT = sum(prof)
    Xoff = np.concatenate([[0], np.cumsum([8 * t for t in prof])])

    x_bf = x.astype(BF16)
    wp_cache = {}

    def wmat(pid):
        """Merged pair weight in [128, f*1024 + kk*128 + c] layout."""
        if pid not in wp_cache:
            e1, e2 = pid // E, pid % E
            Wp = (0.5 * (We[e1] + We[e2])).astype(BF16)
            wp_cache[pid] = np.ascontiguousarray(
                Wp.reshape(KC, P, FC, P).transpose(1, 2, 0, 3).reshape(P, KC * D)
            )
        return wp_cache[pid]

    xT_cores = np.zeros((NCORES, P, 8 * sumT), dtype=BF16)
    w_cores = np.zeros((NCORES, S, P, KC * D), dtype=BF16)
    for c in range(NCORES):
        for r in range(S):
            pid, toks = grid[c][r]
            if pid < 0:
                continue
            w_cores[c, r] = wmat(pid)
            Tr = prof[r]
            xs = np.zeros((Tr, D), dtype=BF16)
            xs[: len(toks)] = x_bf[toks]
            # [128, kk*Tr + t] = x[tok_t, kk*128 + p]
            blk = xs.reshape(Tr, KC, P).transpose(2, 1, 0).reshape(P, 8 * Tr)
            xT_cores[c, :, Xoff[r] : Xoff[r + 1]] = blk

    key = tuple(prof)
    if key not in _prog_cache:
        _prog_cache[key] = _build_program(prof)
    nc = _prog_cache[key]

    in_maps = [{"xT": xT_cores[c], "w": w_cores[c]} for c in range(NCORES)]
    res = run_bass_kernel_spmd(nc, in_maps, core_ids=list(range(NCORES)))
    _last_results = res

    y = np.zeros((N, D), dtype=np.float32)
    covered = np.zeros(N, dtype=np.int64)
    for c in range(NCORES):
        oc = res.results[c]["out"]
        for r in range(S):
            pid, toks = grid[c][r]
            if pid < 0 or len(toks) == 0:
                continue
            Tr = prof[r]
            blk = oc[:, Xoff[r] : Xoff[r + 1]].reshape(P, FC, Tr)
            ys = blk.transpose(2, 1, 0).reshape(Tr, D)[: len(toks)]
            e1, e2 = pid // E, pid % E
            y[toks] = ys.astype(np.float32) + 0.5 * (be[e1] + be[e2])
            covered[toks] += 1

    assert (covered == 1).all(), "dispatch did not cover every token once"
    return y
